# revision 32
# baseline (speedup 1.0000x reference)
"""Gaussian NLL loss kernel for Trainium2 (8 NeuronCores, data-parallel).

out[n] = 0.5 * (x_n - mu)^T pinv(sigma+eps) (x_n - mu) + log_den,  shape [N, 1]

Strategy (v2 — dtype-shrunk, DMA-roofline focused):
  Host: tiny D x D prep in float64 (pinv -> symmetrize -> Cholesky L,
  slogdet), pre-subtract mu from X, cast X^T to bf16 (halves HBM
  traffic vs fp32; numpy study: max rel err ~7e-3 vs 2e-2 gate).
  Device (per core, N/8 samples), per 8192-sample chunk:
    pass1  z = (L/sqrt 2)^T xc      bf16 matmul, stationary L', 1 cyc/col
    square zsq = z^2 -> fp8e4       split between ScalarE (Square
                                    activation) and VectorE (tensor_mul)
                                    to halve the PSUM-drain time
    pass2  q = colsum(zsq)          fp8e4 DoubleRow matmul: each MM
                                    reduces TWO 512-sample blocks into
                                    disjoint output partitions at 0.5
                                    cyc/row -> 0.25 cyc/sample
    out    q + log_den              ScalarE Copy w/ bias (the required
                                    PSUM->SBUF drain), then DMA out
  Pure data-parallel: no collectives.
"""

import math
import sys

import numpy as np

sys.path.insert(0, "/opt/trn_rl_repo")

import ml_dtypes

import concourse.bass as bass
import concourse.bacc as bacc
import concourse.mybir as mybir
import concourse.tile as tile
from concourse.bass_utils import run_bass_kernel_spmd

N, D = 1048576, 128
NCORES = 8
NSH = N // NCORES   # 131072 samples per core
CHUNK = 8192        # samples per DMA tile (bf16: 16KB per partition line)
GROUP = 1024        # samples per square op (one 2-bank PSUM tile)
SUB = 512           # samples per pass1 matmul (out free dim)
QGRP = 4096         # samples per accumulated pq tile [8, 512]

_f32 = mybir.dt.float32
_bf16 = mybir.dt.bfloat16
_f8 = mybir.dt.float8e4

LAST_RESULTS = None  # BassKernelResults of the most recent run (for test.py)


def _build_bass(log_den: float, nsh: int) -> bass.Bass:
    nc = bacc.Bacc()
    xt = nc.declare_dram_parameter("xt", [D, nsh], _bf16, isOutput=False)
    lw = nc.declare_dram_parameter("lw", [D, D], _bf16, isOutput=False)
    sw = nc.declare_dram_parameter("sw", [D, D], _bf16, isOutput=False)
    sel = nc.declare_dram_parameter("sel", [D, 128], _f8, isOutput=False)
    out = nc.declare_dram_parameter("out", [nsh // CHUNK, CHUNK], _f32,
                                    isOutput=True)

    n_chunks = nsh // CHUNK
    n_groups = CHUNK // GROUP           # 8 drain ops per chunk
    # Interleave the two drain engines so they run concurrently and the pz
    # pool never backs up behind one engine's burst.
    act_set = {0, 2, 3, 5, 6}           # ScalarE squares; DVE takes 1, 4, 7

    with tile.TileContext(nc) as tc:
        with (
            tc.tile_pool(name="const", bufs=1) as cpool,
            tc.tile_pool(name="xin", bufs=4) as xpool,
            tc.tile_pool(name="zsq", bufs=2) as zpool,
            tc.tile_pool(name="outs", bufs=3) as opool,
            tc.tile_pool(name="pz", bufs=3, space=bass.MemorySpace.PSUM) as pzpool,
            tc.tile_pool(name="pq", bufs=1, space=bass.MemorySpace.PSUM) as pqpool,
        ):
            lw_t = cpool.tile([D, D], _bf16)
            sw_t = cpool.tile([D, D], _bf16)
            sel_t = cpool.tile([D, 128], _f8)
            nc.sync.dma_start(out=lw_t[:], in_=lw[:])
            nc.sync.dma_start(out=sw_t[:], in_=sw[:])
            nc.sync.dma_start(out=sel_t[:], in_=sel[:])
            # Warm the PE on the const DMAs with a throwaway matmul so real
            # matmuls need only one sync wait.
            warm = pzpool.tile([8, 64], _f32, tag="pz")
            nc.tensor.matmul(warm[:], sel_t[:, 0:8], sel_t[:, 0:64])

            def emit_pass2(i, zq):
                """Colsum + epilogue + output DMA for chunk i's zq.

                Called one iteration late so the in-order PE never stalls
                waiting for the current chunk's ACT/DVE drains.
                """
                pq = pqpool.tile([8, 2 * SUB], _f32, tag="pq")
                for q in range(CHUNK // QGRP):
                    for j in range(4):
                        base = q * QGRP + j * (2 * SUB)
                        # DoubleRow LDW ISA rule (s3_lw_dual_fp8): the lowered
                        # pair stride must be 0 mod 16 elements -> lay each
                        # sel_j out as [2, 16] blocks and use the first 8
                        # columns of each half.
                        nc.tensor.matmul(
                            pq[:, q * SUB:(q + 1) * SUB],
                            sel_t[:, j * 32:(j + 1) * 32].rearrange(
                                "p (two m) -> p two m", two=2
                            )[:, :, 0:8],
                            zq[:, base:base + 2 * SUB].rearrange(
                                "p (two n) -> p two n", two=2
                            ),
                            start=(j == 0),
                            stop=(j == 3),
                            perf_mode=mybir.MatmulPerfMode.DoubleRow,
                            skip_group_check=True,
                        )
                o_t = opool.tile([8, 2 * SUB], _f32)
                # (GpSimd cannot access PSUM on TRN2 -> epilogue on DVE.)
                nc.vector.tensor_scalar_add(o_t[:], pq[:], float(log_den))
                # out row layout: flat sample s = i*CHUNK + q*QGRP + m*SUB + n
                # lives at o_t[m, q*SUB + n].
                for q in range(CHUNK // QGRP):
                    nc.sync.dma_start(
                        out=out[i:i + 1, q * QGRP:(q + 1) * QGRP].rearrange(
                            "a (m n) -> (a m) n", m=8
                        ),
                        in_=o_t[:, q * SUB:(q + 1) * SUB],
                    )

            prev = None
            for i in range(n_chunks):
                x_t = xpool.tile([D, CHUNK], _bf16)
                # Alternate the two HWDGE rings (SP + ACT) so descriptor
                # dispatch is not single-ring bound. Chunk 0 is posted in
                # group-sized pieces so the first pass1 matmul starts after
                # ~1.3us of transfer instead of ~6us.
                dma_eng = nc.sync if (i % 2 == 0) else nc.scalar
                if i == 0:
                    for g in range(n_groups):
                        nc.sync.dma_start(
                            out=x_t[:, g * GROUP:(g + 1) * GROUP],
                            in_=xt[:, g * GROUP:(g + 1) * GROUP],
                        )
                else:
                    dma_eng.dma_start(
                        out=x_t[:], in_=xt[:, i * CHUNK:(i + 1) * CHUNK]
                    )
                zq = zpool.tile([D, CHUNK], _f8)
                for g in range(n_groups):
                    on_act = g in act_set
                    pz = pzpool.tile([D, GROUP], _f32)
                    for s in range(GROUP // SUB):
                        lo = g * GROUP + s * SUB
                        # ACT groups: z = L'^T x, then Square.
                        # DVE groups: y = (S/2)^T x, then x*y elementwise
                        # (one PSUM read; colsum(x*y) == colsum(z^2) == q).
                        nc.tensor.matmul(
                            pz[:, s * SUB:(s + 1) * SUB],
                            lw_t[:] if on_act else sw_t[:],
                            x_t[:, lo:lo + SUB],
                        )
                    dst = zq[:, g * GROUP:(g + 1) * GROUP]
                    if on_act:
                        nc.scalar.activation(
                            dst, pz[:],
                            mybir.ActivationFunctionType.Square,
                        )
                    else:
                        nc.vector.tensor_mul(
                            dst, pz[:],
                            x_t[:, g * GROUP:(g + 1) * GROUP],
                        )
                if prev is not None:
                    emit_pass2(*prev)
                prev = (i, zq)
            emit_pass2(*prev)
    nc.compile()
    return nc


def _host_prep(X, mu, sigma, eps):
    """float64 D x D prep + data-plane casts. Returns (log_den, Lbf, sel_np,
    XT_bf16 [D, N])."""
    sig = sigma.astype(np.float64) + eps.astype(np.float64)
    S = np.linalg.pinv(sig)
    _, logdet = np.linalg.slogdet(sig)
    log_den = 0.5 * (D * math.log(2.0 * math.pi) + logdet)
    Ssym = 0.5 * (S + S.T)
    L = np.linalg.cholesky(Ssym)               # S = L @ L.T
    Lp = (L / math.sqrt(2.0)).astype(np.float32)   # q = ||Lp.T diff||^2
    Lbf = Lp.astype(ml_dtypes.bfloat16)
    Swbf = (0.5 * Ssym).astype(np.float32).astype(ml_dtypes.bfloat16)

    # sel_j for the DoubleRow colsum: w[p, i, m] = 1 iff m == 2j+i, laid out
    # as sel[:, j*32 + i*16 + m] (pair stride 16 to satisfy the DoubleRow
    # LDW ISA restriction).
    sel_np = np.zeros((D, 128), dtype=ml_dtypes.float8_e4m3)
    for j in range(4):
        for i in range(2):
            sel_np[:, j * 32 + i * 16 + (2 * j + i)] = 1.0

    XC = X - mu[None, :].astype(np.float32)
    XT = np.ascontiguousarray(XC.T).astype(ml_dtypes.bfloat16)  # [D, N]
    return log_den, Lbf, Swbf, sel_np, XT


def _install_trace_shim():
    """The image lacks ``antenv.axon_hooks``; recreate it and register the
    ctypes NTFF hook that trn_boot would have installed."""
    import types
    import antenv

    if "antenv.axon_hooks" not in sys.modules:
        mod = types.ModuleType("antenv.axon_hooks")
        holder = [None]
        mod.set_axon_ntff_profile_hook = lambda h: holder.__setitem__(0, h)
        mod.get_axon_ntff_profile_hook = lambda: holder[0]
        sys.modules["antenv.axon_hooks"] = mod
        antenv.axon_hooks = mod
    from antenv.axon_hooks import (
        get_axon_ntff_profile_hook,
        set_axon_ntff_profile_hook,
    )

    if get_axon_ntff_profile_hook() is None:
        from trn_agent_boot.trn_boot import _ntff_profile_via_ctypes

        set_axon_ntff_profile_hook(
            _ntff_profile_via_ctypes("/opt/axon/libaxon_pjrt.so")
        )


def kernel(X: np.ndarray, mu: np.ndarray, sigma: np.ndarray, eps: np.ndarray,
           _trace: bool = False) -> np.ndarray:
    global LAST_RESULTS

    log_den, Lbf, Swbf, sel_np, XT = _host_prep(X, mu, sigma, eps)

    in_maps = []
    for c_id in range(NCORES):
        in_maps.append({
            "xt": np.ascontiguousarray(XT[:, c_id * NSH:(c_id + 1) * NSH]),
            "lw": Lbf,
            "sw": Swbf,
            "sel": sel_np,
        })

    nc = _build_bass(log_den, NSH)
    if _trace:
        _install_trace_shim()
        import tempfile
        import concourse.bass_utils as _bu
        _bu.upload_artifacts = lambda d: "local://" + d  # no S3 in container
        tmpdir = tempfile.mkdtemp(prefix="bass_trace_")
        print("trace dir:", tmpdir)
        res = run_bass_kernel_spmd(
            nc, in_maps, list(range(NCORES)), trace=True, tmpdir=tmpdir
        )
    else:
        res = run_bass_kernel_spmd(nc, in_maps, list(range(NCORES)))
    LAST_RESULTS = res

    out = np.empty((N, 1), dtype=np.float32)
    for c_id in range(NCORES):
        out[c_id * NSH:(c_id + 1) * NSH, 0] = res.results[c_id]["out"].reshape(-1)
    return out


def _sim_selfcheck():
    """CoreSim numerics check on a reduced shard size."""
    import concourse.bass_interp as bass_interp

    nsh = CHUNK * 2
    rng = np.random.default_rng(0)
    X = rng.standard_normal((nsh, D), dtype=np.float32)
    mu = ((rng.random(D, dtype=np.float32) - 0.5) * 2.0)
    A = rng.standard_normal((D, D), dtype=np.float32) * 0.05
    sigma = np.eye(D, dtype=np.float32) + A @ A.T
    eps = (rng.random((D, D), dtype=np.float32) - 0.5) * 1e-15

    log_den, Lbf, Swbf, sel_np, XT = _host_prep(X, mu, sigma, eps)
    nc = _build_bass(log_den, nsh)
    sim = bass_interp.CoreSim(nc)
    sim.tensor("xt")[:] = XT
    sim.tensor("lw")[:] = Lbf
    sim.tensor("sw")[:] = Swbf
    sim.tensor("sel")[:] = sel_np
    sim.simulate()
    got = np.asarray(sim.tensor("out")).reshape(-1).astype(np.float64)

    # reference in numpy float64
    sig = sigma.astype(np.float64) + eps.astype(np.float64)
    S = np.linalg.pinv(sig)
    diff = X.astype(np.float64) - mu.astype(np.float64)
    quad = np.einsum("nd,de,ne->n", diff, S, diff)
    exp = 0.5 * quad + log_den
    rel = np.abs(got - exp) / np.maximum(np.abs(exp), 1e-6)
    print(f"sim rel err: max={rel.max():.3e} mean={rel.mean():.3e}")
    assert rel.max() < 2e-2, rel.max()
    print("SIM PASS")


if __name__ == "__main__":
    if "--sim" in sys.argv:
        _sim_selfcheck()


# revision 33
# speedup vs baseline: 1.0959x; 1.0959x over previous
"""Gaussian NLL loss kernel for Trainium2 (8 NeuronCores, data-parallel).

out[n] = 0.5 * (x_n - mu)^T pinv(sigma+eps) (x_n - mu) + log_den,  shape [N, 1]

Strategy (v2 — dtype-shrunk, DMA-roofline focused):
  Host: tiny D x D prep in float64 (pinv -> symmetrize -> Cholesky L,
  slogdet), pre-subtract mu from X, cast X^T to bf16 (halves HBM
  traffic vs fp32; numpy study: max rel err ~7e-3 vs 2e-2 gate).
  Device (per core, N/8 samples), per 8192-sample chunk:
    pass1  z = (L/sqrt 2)^T xc      bf16 matmul, stationary L', 1 cyc/col
    square zsq = z^2 -> fp8e4       split between ScalarE (Square
                                    activation) and VectorE (tensor_mul)
                                    to halve the PSUM-drain time
    pass2  q = colsum(zsq)          fp8e4 DoubleRow matmul: each MM
                                    reduces TWO 512-sample blocks into
                                    disjoint output partitions at 0.5
                                    cyc/row -> 0.25 cyc/sample
    out    q + log_den              ScalarE Copy w/ bias (the required
                                    PSUM->SBUF drain), then DMA out
  Pure data-parallel: no collectives.
"""

import math
import sys

import numpy as np

sys.path.insert(0, "/opt/trn_rl_repo")

import ml_dtypes

import concourse.bass as bass
import concourse.bacc as bacc
import concourse.mybir as mybir
import concourse.tile as tile
from concourse.bass_utils import run_bass_kernel_spmd

N, D = 1048576, 128
NCORES = 8
NSH = N // NCORES   # 131072 samples per core
CHUNK = 8192        # samples per DMA tile (bf16: 16KB per partition line)
GROUP = 1024        # samples per square op (one 2-bank PSUM tile)
SUB = 512           # samples per pass1 matmul (out free dim)
QGRP = 4096         # samples per accumulated pq tile [8, 512]

_f32 = mybir.dt.float32
_bf16 = mybir.dt.bfloat16
_f8 = mybir.dt.float8e4

LAST_RESULTS = None  # BassKernelResults of the most recent run (for test.py)


def _build_bass(log_den: float, nsh: int) -> bass.Bass:
    nc = bacc.Bacc()
    xt = nc.declare_dram_parameter("xt", [D, nsh], _bf16, isOutput=False)
    lw = nc.declare_dram_parameter("lw", [D, D], _bf16, isOutput=False)
    sw = nc.declare_dram_parameter("sw", [D, D], _bf16, isOutput=False)
    sel = nc.declare_dram_parameter("sel", [D, 128], _f8, isOutput=False)
    out = nc.declare_dram_parameter("out", [nsh // CHUNK, CHUNK], _f32,
                                    isOutput=True)

    n_chunks = nsh // CHUNK
    n_groups = CHUNK // GROUP           # 8 drain ops per chunk
    # Contiguous drain-engine assignment measured faster than interleaved
    # (fewer lw/sw stationary switches on the PE critical path).
    act_set = {0, 1, 2, 3, 4}           # ScalarE squares; DVE takes 5..7

    with tile.TileContext(nc) as tc:
        with (
            tc.tile_pool(name="const", bufs=1) as cpool,
            tc.tile_pool(name="xin", bufs=4) as xpool,
            tc.tile_pool(name="zsq", bufs=2) as zpool,
            tc.tile_pool(name="outs", bufs=3) as opool,
            tc.tile_pool(name="pz", bufs=3, space=bass.MemorySpace.PSUM) as pzpool,
            tc.tile_pool(name="pq", bufs=1, space=bass.MemorySpace.PSUM) as pqpool,
        ):
            lw_t = cpool.tile([D, D], _bf16)
            sw_t = cpool.tile([D, D], _bf16)
            sel_t = cpool.tile([D, 128], _f8)
            nc.sync.dma_start(out=lw_t[:], in_=lw[:])
            nc.sync.dma_start(out=sw_t[:], in_=sw[:])
            nc.sync.dma_start(out=sel_t[:], in_=sel[:])
            # Warm the PE on the const DMAs with a throwaway matmul so real
            # matmuls need only one sync wait.
            warm = pzpool.tile([8, 64], _f32, tag="pz")
            nc.tensor.matmul(warm[:], sel_t[:, 0:8], sel_t[:, 0:64])

            def emit_pass2(i, zq):
                """Colsum + epilogue + output DMA for chunk i's zq.

                Called one iteration late so the in-order PE never stalls
                waiting for the current chunk's ACT/DVE drains.
                """
                pq = pqpool.tile([8, 2 * SUB], _f32, tag="pq")
                for q in range(CHUNK // QGRP):
                    for j in range(4):
                        base = q * QGRP + j * (2 * SUB)
                        # DoubleRow LDW ISA rule (s3_lw_dual_fp8): the lowered
                        # pair stride must be 0 mod 16 elements -> lay each
                        # sel_j out as [2, 16] blocks and use the first 8
                        # columns of each half.
                        nc.tensor.matmul(
                            pq[:, q * SUB:(q + 1) * SUB],
                            sel_t[:, j * 32:(j + 1) * 32].rearrange(
                                "p (two m) -> p two m", two=2
                            )[:, :, 0:8],
                            zq[:, base:base + 2 * SUB].rearrange(
                                "p (two n) -> p two n", two=2
                            ),
                            start=(j == 0),
                            stop=(j == 3),
                            perf_mode=mybir.MatmulPerfMode.DoubleRow,
                            skip_group_check=True,
                        )
                o_t = opool.tile([8, 2 * SUB], _f32)
                # (GpSimd cannot access PSUM on TRN2 -> epilogue on DVE.)
                nc.vector.tensor_scalar_add(o_t[:], pq[:], float(log_den))
                # out row layout: flat sample s = i*CHUNK + q*QGRP + m*SUB + n
                # lives at o_t[m, q*SUB + n].
                for q in range(CHUNK // QGRP):
                    nc.sync.dma_start(
                        out=out[i:i + 1, q * QGRP:(q + 1) * QGRP].rearrange(
                            "a (m n) -> (a m) n", m=8
                        ),
                        in_=o_t[:, q * SUB:(q + 1) * SUB],
                    )

            prev = None
            for i in range(n_chunks):
                x_t = xpool.tile([D, CHUNK], _bf16)
                # Alternate the two HWDGE rings (SP + ACT) so descriptor
                # dispatch is not single-ring bound. Chunk 0 is posted in
                # group-sized pieces so the first pass1 matmul starts after
                # ~1.3us of transfer instead of ~6us.
                dma_eng = nc.sync if (i % 2 == 0) else nc.scalar
                if i == 0:
                    for g in range(n_groups):
                        nc.sync.dma_start(
                            out=x_t[:, g * GROUP:(g + 1) * GROUP],
                            in_=xt[:, g * GROUP:(g + 1) * GROUP],
                        )
                else:
                    dma_eng.dma_start(
                        out=x_t[:], in_=xt[:, i * CHUNK:(i + 1) * CHUNK]
                    )
                zq = zpool.tile([D, CHUNK], _f8)
                for g in range(n_groups):
                    on_act = g in act_set
                    pz = pzpool.tile([D, GROUP], _f32)
                    for s in range(GROUP // SUB):
                        lo = g * GROUP + s * SUB
                        # ACT groups: z = L'^T x, then Square.
                        # DVE groups: y = (S/2)^T x, then x*y elementwise
                        # (one PSUM read; colsum(x*y) == colsum(z^2) == q).
                        nc.tensor.matmul(
                            pz[:, s * SUB:(s + 1) * SUB],
                            lw_t[:] if on_act else sw_t[:],
                            x_t[:, lo:lo + SUB],
                        )
                    dst = zq[:, g * GROUP:(g + 1) * GROUP]
                    if on_act:
                        nc.scalar.activation(
                            dst, pz[:],
                            mybir.ActivationFunctionType.Square,
                        )
                    else:
                        nc.vector.tensor_mul(
                            dst, pz[:],
                            x_t[:, g * GROUP:(g + 1) * GROUP],
                        )
                if prev is not None:
                    emit_pass2(*prev)
                prev = (i, zq)
            emit_pass2(*prev)
    nc.compile()
    return nc


def _host_prep(X, mu, sigma, eps):
    """float64 D x D prep + data-plane casts. Returns (log_den, Lbf, sel_np,
    XT_bf16 [D, N])."""
    sig = sigma.astype(np.float64) + eps.astype(np.float64)
    S = np.linalg.pinv(sig)
    _, logdet = np.linalg.slogdet(sig)
    log_den = 0.5 * (D * math.log(2.0 * math.pi) + logdet)
    Ssym = 0.5 * (S + S.T)
    L = np.linalg.cholesky(Ssym)               # S = L @ L.T
    Lp = (L / math.sqrt(2.0)).astype(np.float32)   # q = ||Lp.T diff||^2
    Lbf = Lp.astype(ml_dtypes.bfloat16)
    Swbf = (0.5 * Ssym).astype(np.float32).astype(ml_dtypes.bfloat16)

    # sel_j for the DoubleRow colsum: w[p, i, m] = 1 iff m == 2j+i, laid out
    # as sel[:, j*32 + i*16 + m] (pair stride 16 to satisfy the DoubleRow
    # LDW ISA restriction).
    sel_np = np.zeros((D, 128), dtype=ml_dtypes.float8_e4m3)
    for j in range(4):
        for i in range(2):
            sel_np[:, j * 32 + i * 16 + (2 * j + i)] = 1.0

    XC = X - mu[None, :].astype(np.float32)
    XT = np.ascontiguousarray(XC.T).astype(ml_dtypes.bfloat16)  # [D, N]
    return log_den, Lbf, Swbf, sel_np, XT


def _install_trace_shim():
    """The image lacks ``antenv.axon_hooks``; recreate it and register the
    ctypes NTFF hook that trn_boot would have installed."""
    import types
    import antenv

    if "antenv.axon_hooks" not in sys.modules:
        mod = types.ModuleType("antenv.axon_hooks")
        holder = [None]
        mod.set_axon_ntff_profile_hook = lambda h: holder.__setitem__(0, h)
        mod.get_axon_ntff_profile_hook = lambda: holder[0]
        sys.modules["antenv.axon_hooks"] = mod
        antenv.axon_hooks = mod
    from antenv.axon_hooks import (
        get_axon_ntff_profile_hook,
        set_axon_ntff_profile_hook,
    )

    if get_axon_ntff_profile_hook() is None:
        from trn_agent_boot.trn_boot import _ntff_profile_via_ctypes

        set_axon_ntff_profile_hook(
            _ntff_profile_via_ctypes("/opt/axon/libaxon_pjrt.so")
        )


def kernel(X: np.ndarray, mu: np.ndarray, sigma: np.ndarray, eps: np.ndarray,
           _trace: bool = False) -> np.ndarray:
    global LAST_RESULTS

    log_den, Lbf, Swbf, sel_np, XT = _host_prep(X, mu, sigma, eps)

    in_maps = []
    for c_id in range(NCORES):
        in_maps.append({
            "xt": np.ascontiguousarray(XT[:, c_id * NSH:(c_id + 1) * NSH]),
            "lw": Lbf,
            "sw": Swbf,
            "sel": sel_np,
        })

    nc = _build_bass(log_den, NSH)
    if _trace:
        _install_trace_shim()
        import tempfile
        import concourse.bass_utils as _bu
        _bu.upload_artifacts = lambda d: "local://" + d  # no S3 in container
        tmpdir = tempfile.mkdtemp(prefix="bass_trace_")
        print("trace dir:", tmpdir)
        res = run_bass_kernel_spmd(
            nc, in_maps, list(range(NCORES)), trace=True, tmpdir=tmpdir
        )
    else:
        res = run_bass_kernel_spmd(nc, in_maps, list(range(NCORES)))
    LAST_RESULTS = res

    out = np.empty((N, 1), dtype=np.float32)
    for c_id in range(NCORES):
        out[c_id * NSH:(c_id + 1) * NSH, 0] = res.results[c_id]["out"].reshape(-1)
    return out


def _sim_selfcheck():
    """CoreSim numerics check on a reduced shard size."""
    import concourse.bass_interp as bass_interp

    nsh = CHUNK * 2
    rng = np.random.default_rng(0)
    X = rng.standard_normal((nsh, D), dtype=np.float32)
    mu = ((rng.random(D, dtype=np.float32) - 0.5) * 2.0)
    A = rng.standard_normal((D, D), dtype=np.float32) * 0.05
    sigma = np.eye(D, dtype=np.float32) + A @ A.T
    eps = (rng.random((D, D), dtype=np.float32) - 0.5) * 1e-15

    log_den, Lbf, Swbf, sel_np, XT = _host_prep(X, mu, sigma, eps)
    nc = _build_bass(log_den, nsh)
    sim = bass_interp.CoreSim(nc)
    sim.tensor("xt")[:] = XT
    sim.tensor("lw")[:] = Lbf
    sim.tensor("sw")[:] = Swbf
    sim.tensor("sel")[:] = sel_np
    sim.simulate()
    got = np.asarray(sim.tensor("out")).reshape(-1).astype(np.float64)

    # reference in numpy float64
    sig = sigma.astype(np.float64) + eps.astype(np.float64)
    S = np.linalg.pinv(sig)
    diff = X.astype(np.float64) - mu.astype(np.float64)
    quad = np.einsum("nd,de,ne->n", diff, S, diff)
    exp = 0.5 * quad + log_den
    rel = np.abs(got - exp) / np.maximum(np.abs(exp), 1e-6)
    print(f"sim rel err: max={rel.max():.3e} mean={rel.mean():.3e}")
    assert rel.max() < 2e-2, rel.max()
    print("SIM PASS")


if __name__ == "__main__":
    if "--sim" in sys.argv:
        _sim_selfcheck()
